# revision 17
# baseline (speedup 1.0000x reference)
"""Trainium2 Bass kernel for nn_InterfaceGraph (retrieval_knn).

Value-only formulation: the reference's outputs (mask, dists) depend only
on each atom's MINIMUM same-graph distance, not on which neighbor attains
it.  So the device computes, per atom, max_j key_ij where key = -d^2 is
produced directly by one bf16 matmul per 128-row tile (K=36 rows: per
coordinate, split3 cross terms plus |a_c|^2 / |b_c|^2 rows, ordered so
fp32 PSUM accumulation cancels early; max abs d^2 error 0.014 on the
target data).  VectorE does ONE slab tensor_reduce(max) per 4-bank PSUM
group -- no FIND_INDEX8 / MAX8 passes at all, which removes ~60% of the
baseline's DVE time.

Host epilogue: d = sqrt(-max); rows with d < 2.5 or |d-10| < 0.6 (~11%)
are recomputed exactly (fp64 argmin + fp32 norm, matching the reference
formula) so small-d relative error and the d<10 interface-cutoff
comparisons are exact; residue segment-max mask + mutation OR as before.

Sharding: all 316 row-tiles (both directions) are sorted by column width
and dealt round-robin to the 8 cores, so per-slot cross-core widths are
tight (SPMD program shapes are cross-core maxima).
"""

import numpy as np


NCORES = 8
G = 64
NUM_RESIDUES = 2048
CUTOFF = np.float32(10.0)
BIG = np.float32(60000.0)   # fp16-representable; real keys stay > -6000
K = 15
GROUP = 4          # psum banks (tiles) per reduce slab

PROFILE = False
LAST_EXEC_NS = None

F16 = np.float16

_prog_cache = {}


def _round_up(x, m):
    return (x + m - 1) // m * m


def _install_ntff_hook():
    import sys
    import types
    if 'antenv.axon_hooks' in sys.modules:
        return
    from trn_agent_boot.trn_boot import _ntff_profile_via_ctypes
    hook = _ntff_profile_via_ctypes('/opt/axon/libaxon_pjrt.so')
    mod = types.ModuleType('antenv.axon_hooks')
    mod.get_axon_ntff_profile_hook = lambda: hook
    sys.modules['antenv.axon_hooks'] = mod


def _split2(v):
    v = v.astype(np.float32)
    v1 = v.astype(F16).astype(np.float32)
    v2 = (v - v1).astype(F16).astype(np.float32)
    return v1, v2


class _Geom:
    """Tile lists and per-slot shapes for one side (row->col direction).

    A tile is 128 consecutive row-atoms of one graph scanning that
    graph's full column block.  Tiles from BOTH sides... (this class is
    one side; kernel builds two).  Sorted by padded column width desc,
    slot s holds tiles [8s:8s+8] across the 8 cores; missing entries are
    dummy tiles (zero lhs).
    """

    def __init__(self, n_row, n_col):
        tiles = []          # (graph, row_chunk, W)
        for g in range(G):
            W = max(8, _round_up(int(n_col[g]), 4))
            for r in range(-(-int(n_row[g]) // 128)):
                tiles.append((g, r, W))
        tiles.sort(key=lambda t: (-t[2], t[0], t[1]))
        self.nslots = -(-len(tiles) // NCORES)
        # pad to full slots with dummies (graph=-1)
        tiles += [(-1, 0, 8)] * (self.nslots * NCORES - len(tiles))
        self.ngroups = -(-self.nslots // GROUP)
        tiles += [(-1, 0, 8)] * ((self.ngroups * GROUP - self.nslots) * NCORES)
        self.nslots = self.ngroups * GROUP
        # slot s, core c -> tiles[s*8 + c]
        self.tile = [[tiles[s * NCORES + c] for c in range(NCORES)]
                     for s in range(self.nslots)]
        self.Wgrp = []
        for grp in range(self.ngroups):
            w = max(self.tile[s][c][2]
                    for s in range(grp * GROUP,
                                   min((grp + 1) * GROUP, self.nslots))
                    for c in range(NCORES))
            self.Wgrp.append(int(_round_up(w, 4)))
        self.L = self.nslots * 128                 # lhs columns
        # rhs windows: dedup per (graph, Wgrp) per core
        self.rhs_cols = [GROUP * w for w in self.Wgrp]
        self.rhs_base = np.concatenate(
            [[0], np.cumsum(self.rhs_cols)]).astype(int)
        self.R = int(self.rhs_base[-1])

    def key(self):
        return (self.nslots, tuple(self.Wgrp))


def _build_program(gA, gB):
    from contextlib import ExitStack

    import concourse.bacc as bacc
    import concourse.mybir as mybir
    import concourse.tile as tile

    f32 = mybir.dt.float32
    f16 = mybir.dt.float16

    nc = bacc.Bacc("TRN2", target_bir_lowering=False, debug=False,
                   enable_asserts=True, num_devices=NCORES)

    from concourse import masks

    bf16 = mybir.dt.bfloat16

    lhsA = nc.dram_tensor("lhsA", [K, gA.L], f16, kind="ExternalInput").ap()
    rhsA = nc.dram_tensor("rhsA", [K, gA.R], f16, kind="ExternalInput").ap()
    lhsB = nc.dram_tensor("lhsB", [K, gB.L], f16, kind="ExternalInput").ap()
    rhsB = nc.dram_tensor("rhsB", [K, gB.R], f16, kind="ExternalInput").ap()
    valA = nc.dram_tensor("valA", [gA.nslots, 128], f32,
                          kind="ExternalOutput").ap()
    valB = nc.dram_tensor("valB", [gB.nslots, 128], f32,
                          kind="ExternalOutput").ap()

    with tile.TileContext(nc) as tc:
        with ExitStack() as ctx:
            const = ctx.enter_context(tc.tile_pool(name="const", bufs=1))
            psum = ctx.enter_context(
                tc.tile_pool(name="psum", bufs=2, space="PSUM"))
            work = ctx.enter_context(tc.tile_pool(name="work", bufs=4))

            lhsA_sb = const.tile([K, gA.L], f16, tag="lhsA")
            rhsA_sb = const.tile([K, gA.R], f16, tag="rhsA")
            lhsB_sb = const.tile([K, gB.L], f16, tag="lhsB")
            rhsB_sb = const.tile([K, gB.R], f16, tag="rhsB")
            valA_sb = const.tile([128, gA.nslots], f32, tag="valA")
            valB_sb = const.tile([128, gB.nslots], f32, tag="valB")
            valTA_sb = const.tile([gA.nslots, 128], f32, tag="valTA")
            valTB_sb = const.tile([gB.nslots, 128], f32, tag="valTB")
            ident = const.tile([128, 128], f32, tag="ident")
            masks.make_identity(nc, ident[:])

            # Staged input DMAs on separate queues: group-0 slices first
            # (gate the first matmuls), bulk behind them.  Vector engine
            # issues nothing (it is the compute bottleneck).
            cut_lA = min(GROUP, gA.nslots) * 128
            cut_rA = int(gA.rhs_base[1])
            cut_lB = min(GROUP, gB.nslots) * 128
            cut_rB = int(gB.rhs_base[1])
            nc.sync.dma_start(rhsA_sb[:, :cut_rA], rhsA[:, :cut_rA])
            nc.scalar.dma_start(lhsA_sb[:, :cut_lA], lhsA[:, :cut_lA])
            nc.gpsimd.dma_start(rhsA_sb[:, cut_rA:], rhsA[:, cut_rA:])
            nc.sync.dma_start(lhsA_sb[:, cut_lA:], lhsA[:, cut_lA:])
            nc.scalar.dma_start(rhsB_sb[:, :cut_rB], rhsB[:, :cut_rB])
            nc.gpsimd.dma_start(lhsB_sb[:, :cut_lB], lhsB[:, :cut_lB])
            nc.sync.dma_start(rhsB_sb[:, cut_rB:], rhsB[:, cut_rB:])
            nc.scalar.dma_start(lhsB_sb[:, cut_lB:], lhsB[:, cut_lB:])

            def side(geom, lhs_sb, rhs_sb, val_sb, valT_sb, valT):
                for grp in range(geom.ngroups):
                    W = geom.Wgrp[grp]
                    ps = psum.tile([128, GROUP, 512], f32, tag="ps")
                    for k in range(GROUP):
                        s = grp * GROUP + k
                        off = int(geom.rhs_base[grp]) + k * W
                        nc.tensor.matmul(
                            ps[:, k, 0:W],
                            lhs_sb[:, s * 128:(s + 1) * 128],
                            rhs_sb[:, off:off + W],
                            start=True, stop=True)
                    nc.vector.reduce_max(
                        val_sb[:, grp * GROUP:(grp + 1) * GROUP],
                        ps[:, :, 0:W], axis=mybir.AxisListType.X)
                # transpose [128, nslots] -> [nslots, 128] so the output DMA
                # uses nslots fat descriptors instead of 128 x tiny ones
                pst = psum.tile([128, GROUP, 512], f32, tag="ps")
                nc.tensor.transpose(
                    pst[0:geom.nslots, 0, 0:128], val_sb[:], ident[:])
                nc.scalar.activation(
                    valT_sb[:], pst[0:geom.nslots, 0, 0:128],
                    mybir.ActivationFunctionType.Copy)
                nc.sync.dma_start(valT[:], valT_sb[:])

            side(gA, lhsA_sb, rhsA_sb, valA_sb, valTA_sb, valA)
            side(gB, lhsB_sb, rhsB_sb, valB_sb, valTB_sb, valB)

    nc.compile()
    return nc


def _pack_side(geom, pos_row, pos_col, starts_row, starts_col, core):
    """lhs [K, L] / rhs [K, R] bf16 for one core, one side."""
    lhs = np.zeros((K, geom.L), np.float32)
    rhs = np.zeros((K, geom.R), np.float32)
    rhs[1, :] = BIG                    # default: every rhs col loses the max
    for grp in range(geom.ngroups):
        W = geom.Wgrp[grp]
        for k in range(GROUP):
            s = grp * GROUP + k
            g, r, _ = geom.tile[s][core]
            if g < 0:
                continue
            lb = s * 128
            p = pos_row[starts_row[g] + 128 * r:
                        min(starts_row[g] + 128 * (r + 1), starts_row[g + 1])]
            n = p.shape[0]
            off = int(geom.rhs_base[grp]) + k * W
            q = pos_col[starts_col[g]:starts_col[g + 1]]
            m = q.shape[0]
            for c in range(3):
                base = c * 5
                u1, u2 = _split2(q[:, c])
                v1, v2 = _split2(q[:, c] * q[:, c])
                sl = slice(off, off + m)
                rhs[base + 0, sl] = u1
                rhs[base + 1, sl] = v1
                rhs[base + 2, sl] = u2
                rhs[base + 3, sl] = u1
                rhs[base + 4, sl] = v2
            rhs[1, off + m:off + W] = BIG
            for c in range(3):
                base = c * 5
                x1, x2 = _split2(np.float32(2.0) * p[:, c])
                sl = slice(lb, lb + n)
                lhs[base + 0, sl] = x1
                lhs[base + 1, sl] = -1.0
                lhs[base + 2, sl] = x1
                lhs[base + 3, sl] = x2
                lhs[base + 4, sl] = -1.0
    return lhs.astype(F16), rhs.astype(F16)


def kernel(pos_a, pos_b, node2graph_a, node2graph_b,
           atom2residue_a, atom2residue_b, is_mutation):
    global LAST_EXEC_NS

    from concourse.bass_utils import run_bass_kernel_spmd

    pos_a = np.asarray(pos_a, dtype=np.float32)
    pos_b = np.asarray(pos_b, dtype=np.float32)
    node2graph_a = np.asarray(node2graph_a)
    node2graph_b = np.asarray(node2graph_b)
    atom2residue_a = np.asarray(atom2residue_a)
    atom2residue_b = np.asarray(atom2residue_b)
    is_mutation = np.asarray(is_mutation)

    sa = np.searchsorted(node2graph_a, np.arange(G + 1)).astype(np.int64)
    sb = np.searchsorted(node2graph_b, np.arange(G + 1)).astype(np.int64)
    na = np.diff(sa)
    nb = np.diff(sb)
    assert na.min() > 0 and nb.min() > 0, "empty graph block not supported"

    gA = _Geom(na, nb)        # a rows vs b cols
    gB = _Geom(nb, na)        # b rows vs a cols
    key = (gA.key(), gB.key())
    if key not in _prog_cache:
        _prog_cache[key] = _build_program(gA, gB)
    nc = _prog_cache[key]

    in_maps = []
    for c in range(NCORES):
        lhsA, rhsA = _pack_side(gA, pos_a, pos_b, sa, sb, c)
        lhsB, rhsB = _pack_side(gB, pos_b, pos_a, sb, sa, c)
        in_maps.append({"lhsA": lhsA, "rhsA": rhsA,
                        "lhsB": lhsB, "rhsB": rhsB})

    if PROFILE:
        _install_ntff_hook()
    res = run_bass_kernel_spmd(nc, in_maps, list(range(NCORES)),
                               trace=bool(PROFILE))
    if PROFILE:
        LAST_EXEC_NS = res.exec_time_ns

    key_a = np.empty(pos_a.shape[0], np.float64)
    key_b = np.empty(pos_b.shape[0], np.float64)
    for c in range(NCORES):
        vA = res.results[c]["valA"]
        vB = res.results[c]["valB"]
        for s in range(gA.nslots):
            g, r, _ = gA.tile[s][c]
            if g < 0:
                continue
            lo = sa[g] + 128 * r
            hi = min(sa[g] + 128 * (r + 1), sa[g + 1])
            key_a[lo:hi] = vA[s, 0:hi - lo]
        for s in range(gB.nslots):
            g, r, _ = gB.tile[s][c]
            if g < 0:
                continue
            lo = sb[g] + 128 * r
            hi = min(sb[g] + 128 * (r + 1), sb[g + 1])
            key_b[lo:hi] = vB[s, 0:hi - lo]

    # key = max_j (2 a.b_j - |b_j|^2) = |a|^2 - d2_min; |a|^2 exact on host
    d2_a = (pos_a.astype(np.float64) ** 2).sum(-1) - key_a
    d2_b = (pos_b.astype(np.float64) ** 2).sum(-1) - key_b

    def epilogue(d2dev, pos_row, pos_col, s_col, n2row):
        dist = np.sqrt(np.maximum(d2dev, 0.0)).astype(np.float32)
        flags = np.where((dist < 3.0) | (np.abs(dist - 10.0) < 0.6))[0]
        for i in flags:
            g = n2row[i]
            Q = pos_col[s_col[g]:s_col[g + 1]]
            df = pos_row[i].astype(np.float64) - Q.astype(np.float64)
            j = int(np.argmin((df * df).sum(-1)))
            diff = (pos_row[i] - Q[j]).astype(np.float32)
            dist[i] = np.float32(np.sqrt(np.float32((diff * diff).sum())))
        return dist

    dist_a = epilogue(d2_a, pos_a, pos_b, sb, node2graph_a)
    dist_b = epilogue(d2_b, pos_b, pos_a, sa, node2graph_b)

    def iface_mask(dist, atom2residue):
        is_if = (dist < CUTOFF).astype(np.int32)
        res_max = np.zeros(NUM_RESIDUES, dtype=np.int32)
        np.maximum.at(res_max, atom2residue, is_if)
        return res_max[atom2residue] > 0

    mask_a = iface_mask(dist_a, atom2residue_a)
    mask_b = iface_mask(dist_b, atom2residue_b)
    mask = np.concatenate([mask_a, mask_b]) | is_mutation.astype(bool)
    dists = np.concatenate([dist_a, dist_b]).astype(np.float32)
    return mask, dists


# revision 18
# speedup vs baseline: 1.1358x; 1.1358x over previous
"""Trainium2 Bass kernel for nn_InterfaceGraph (retrieval_knn).

Value-only formulation: the reference's outputs (mask, dists) depend only
on each atom's MINIMUM same-graph distance, not on which neighbor attains
it.  So the device computes, per atom, max_j key_ij where key = -d^2 is
produced directly by one bf16 matmul per 128-row tile (K=36 rows: per
coordinate, split3 cross terms plus |a_c|^2 / |b_c|^2 rows, ordered so
fp32 PSUM accumulation cancels early; max abs d^2 error 0.014 on the
target data).  VectorE does ONE slab tensor_reduce(max) per 4-bank PSUM
group -- no FIND_INDEX8 / MAX8 passes at all, which removes ~60% of the
baseline's DVE time.

Host epilogue: d = sqrt(-max); rows with d < 2.5 or |d-10| < 0.6 (~11%)
are recomputed exactly (fp64 argmin + fp32 norm, matching the reference
formula) so small-d relative error and the d<10 interface-cutoff
comparisons are exact; residue segment-max mask + mutation OR as before.

Sharding: all 316 row-tiles (both directions) are sorted by column width
and dealt round-robin to the 8 cores, so per-slot cross-core widths are
tight (SPMD program shapes are cross-core maxima).
"""

import numpy as np


NCORES = 8
G = 64
NUM_RESIDUES = 2048
CUTOFF = np.float32(10.0)
BIG = np.float32(60000.0)   # fp16-representable; real keys stay > -6000
K = 15
GROUP = 4          # psum banks (tiles) per reduce slab

PROFILE = False
LAST_EXEC_NS = None

F16 = np.float16

_prog_cache = {}


def _round_up(x, m):
    return (x + m - 1) // m * m


def _install_ntff_hook():
    import sys
    import types
    if 'antenv.axon_hooks' in sys.modules:
        return
    from trn_agent_boot.trn_boot import _ntff_profile_via_ctypes
    hook = _ntff_profile_via_ctypes('/opt/axon/libaxon_pjrt.so')
    mod = types.ModuleType('antenv.axon_hooks')
    mod.get_axon_ntff_profile_hook = lambda: hook
    sys.modules['antenv.axon_hooks'] = mod


def _split2(v):
    v = v.astype(np.float32)
    v1 = v.astype(F16).astype(np.float32)
    v2 = (v - v1).astype(F16).astype(np.float32)
    return v1, v2


class _Geom:
    """Tile lists and per-slot shapes for one side (row->col direction).

    A tile is 128 consecutive row-atoms of one graph scanning that
    graph's full column block.  Tiles from BOTH sides... (this class is
    one side; kernel builds two).  Sorted by padded column width desc,
    slot s holds tiles [8s:8s+8] across the 8 cores; missing entries are
    dummy tiles (zero lhs).
    """

    def __init__(self, n_row, n_col):
        tiles = []          # (graph, row_chunk, W)
        for g in range(G):
            W = max(8, _round_up(int(n_col[g]), 4))
            for r in range(-(-int(n_row[g]) // 128)):
                tiles.append((g, r, W))
        tiles.sort(key=lambda t: (-t[2], t[0], t[1]))
        self.nslots = -(-len(tiles) // NCORES)
        # pad to full slots with dummies (graph=-1)
        tiles += [(-1, 0, 8)] * (self.nslots * NCORES - len(tiles))
        self.ngroups = -(-self.nslots // GROUP)
        tiles += [(-1, 0, 8)] * ((self.ngroups * GROUP - self.nslots) * NCORES)
        self.nslots = self.ngroups * GROUP
        # slot s, core c -> tiles[s*8 + c]
        self.tile = [[tiles[s * NCORES + c] for c in range(NCORES)]
                     for s in range(self.nslots)]
        self.Wgrp = []
        for grp in range(self.ngroups):
            w = max(self.tile[s][c][2]
                    for s in range(grp * GROUP,
                                   min((grp + 1) * GROUP, self.nslots))
                    for c in range(NCORES))
            self.Wgrp.append(int(_round_up(w, 4)))
        self.L = self.nslots * 128                 # lhs columns
        # rhs windows: dedup per (graph, Wgrp) per core
        self.rhs_cols = [GROUP * w for w in self.Wgrp]
        self.rhs_base = np.concatenate(
            [[0], np.cumsum(self.rhs_cols)]).astype(int)
        self.R = int(self.rhs_base[-1])

    def key(self):
        return (self.nslots, tuple(self.Wgrp))


def _build_program(gA, gB):
    from contextlib import ExitStack

    import concourse.bacc as bacc
    import concourse.mybir as mybir
    import concourse.tile as tile

    f32 = mybir.dt.float32
    f16 = mybir.dt.float16

    nc = bacc.Bacc("TRN2", target_bir_lowering=False, debug=False,
                   enable_asserts=True, num_devices=NCORES)

    from concourse import masks

    bf16 = mybir.dt.bfloat16

    lhsA = nc.dram_tensor("lhsA", [K, gA.L], f16, kind="ExternalInput").ap()
    rhsA = nc.dram_tensor("rhsA", [K, gA.R], f16, kind="ExternalInput").ap()
    lhsB = nc.dram_tensor("lhsB", [K, gB.L], f16, kind="ExternalInput").ap()
    rhsB = nc.dram_tensor("rhsB", [K, gB.R], f16, kind="ExternalInput").ap()
    valA = nc.dram_tensor("valA", [gA.nslots, 128], f32,
                          kind="ExternalOutput").ap()
    valB = nc.dram_tensor("valB", [gB.nslots, 128], f32,
                          kind="ExternalOutput").ap()

    with tile.TileContext(nc) as tc:
        with ExitStack() as ctx:
            const = ctx.enter_context(tc.tile_pool(name="const", bufs=1))
            psum = ctx.enter_context(
                tc.tile_pool(name="psum", bufs=2, space="PSUM"))
            work = ctx.enter_context(tc.tile_pool(name="work", bufs=4))

            lhsA_sb = const.tile([K, gA.L], f16, tag="lhsA")
            rhsA_sb = const.tile([K, gA.R], f16, tag="rhsA")
            lhsB_sb = const.tile([K, gB.L], f16, tag="lhsB")
            rhsB_sb = const.tile([K, gB.R], f16, tag="rhsB")
            valA_sb = const.tile([128, gA.nslots], f32, tag="valA")
            valB_sb = const.tile([128, gB.nslots], f32, tag="valB")
            valTA_sb = const.tile([gA.nslots, 128], f32, tag="valTA")
            valTB_sb = const.tile([gB.nslots, 128], f32, tag="valTB")
            ident = const.tile([128, 128], f32, tag="ident")
            masks.make_identity(nc, ident[:])

            # Staged input DMAs on separate queues: group-0 slices first
            # (gate the first matmuls), bulk behind them.  Vector engine
            # issues nothing (it is the compute bottleneck).
            cut_lA = min(GROUP, gA.nslots) * 128
            cut_rA = int(gA.rhs_base[1])
            cut_lB = min(GROUP, gB.nslots) * 128
            cut_rB = int(gB.rhs_base[1])
            nc.sync.dma_start(rhsA_sb[:, :cut_rA], rhsA[:, :cut_rA])
            nc.scalar.dma_start(lhsA_sb[:, :cut_lA], lhsA[:, :cut_lA])
            nc.gpsimd.dma_start(rhsA_sb[:, cut_rA:], rhsA[:, cut_rA:])
            nc.sync.dma_start(lhsA_sb[:, cut_lA:], lhsA[:, cut_lA:])
            nc.scalar.dma_start(rhsB_sb[:, :cut_rB], rhsB[:, :cut_rB])
            nc.gpsimd.dma_start(lhsB_sb[:, :cut_lB], lhsB[:, :cut_lB])
            nc.sync.dma_start(rhsB_sb[:, cut_rB:], rhsB[:, cut_rB:])
            nc.scalar.dma_start(lhsB_sb[:, cut_lB:], lhsB[:, cut_lB:])

            def side(geom, lhs_sb, rhs_sb, val_sb, valT_sb, valT, dma_eng):
                for grp in range(geom.ngroups):
                    W = geom.Wgrp[grp]
                    ps = psum.tile([128, GROUP, 512], f32, tag="ps")
                    for k in range(GROUP):
                        s = grp * GROUP + k
                        off = int(geom.rhs_base[grp]) + k * W
                        nc.tensor.matmul(
                            ps[:, k, 0:W],
                            lhs_sb[:, s * 128:(s + 1) * 128],
                            rhs_sb[:, off:off + W],
                            start=True, stop=True)
                    nc.vector.reduce_max(
                        val_sb[:, grp * GROUP:(grp + 1) * GROUP],
                        ps[:, :, 0:W], axis=mybir.AxisListType.X)
                # transpose [128, nslots] -> [nslots, 128] so the output DMA
                # uses nslots fat descriptors instead of 128 x tiny ones
                pst = psum.tile([128, GROUP, 512], f32, tag="ps")
                nc.tensor.transpose(
                    pst[0:geom.nslots, 0, 0:128], val_sb[:], ident[:])
                nc.vector.tensor_copy(
                    valT_sb[:], pst[0:geom.nslots, 0, 0:128])
                dma_eng.dma_start(valT[:], valT_sb[:])

            side(gA, lhsA_sb, rhsA_sb, valA_sb, valTA_sb, valA, nc.sync)
            side(gB, lhsB_sb, rhsB_sb, valB_sb, valTB_sb, valB, nc.gpsimd)

    nc.compile()
    return nc


def _pack_side(geom, pos_row, pos_col, starts_row, starts_col, core):
    """lhs [K, L] / rhs [K, R] bf16 for one core, one side."""
    lhs = np.zeros((K, geom.L), np.float32)
    rhs = np.zeros((K, geom.R), np.float32)
    rhs[1, :] = BIG                    # default: every rhs col loses the max
    for grp in range(geom.ngroups):
        W = geom.Wgrp[grp]
        for k in range(GROUP):
            s = grp * GROUP + k
            g, r, _ = geom.tile[s][core]
            if g < 0:
                continue
            lb = s * 128
            p = pos_row[starts_row[g] + 128 * r:
                        min(starts_row[g] + 128 * (r + 1), starts_row[g + 1])]
            n = p.shape[0]
            off = int(geom.rhs_base[grp]) + k * W
            q = pos_col[starts_col[g]:starts_col[g + 1]]
            m = q.shape[0]
            for c in range(3):
                base = c * 5
                u1, u2 = _split2(q[:, c])
                v1, v2 = _split2(q[:, c] * q[:, c])
                sl = slice(off, off + m)
                rhs[base + 0, sl] = u1
                rhs[base + 1, sl] = v1
                rhs[base + 2, sl] = u2
                rhs[base + 3, sl] = u1
                rhs[base + 4, sl] = v2
            rhs[1, off + m:off + W] = BIG
            for c in range(3):
                base = c * 5
                x1, x2 = _split2(np.float32(2.0) * p[:, c])
                sl = slice(lb, lb + n)
                lhs[base + 0, sl] = x1
                lhs[base + 1, sl] = -1.0
                lhs[base + 2, sl] = x1
                lhs[base + 3, sl] = x2
                lhs[base + 4, sl] = -1.0
    return lhs.astype(F16), rhs.astype(F16)


def kernel(pos_a, pos_b, node2graph_a, node2graph_b,
           atom2residue_a, atom2residue_b, is_mutation):
    global LAST_EXEC_NS

    from concourse.bass_utils import run_bass_kernel_spmd

    pos_a = np.asarray(pos_a, dtype=np.float32)
    pos_b = np.asarray(pos_b, dtype=np.float32)
    node2graph_a = np.asarray(node2graph_a)
    node2graph_b = np.asarray(node2graph_b)
    atom2residue_a = np.asarray(atom2residue_a)
    atom2residue_b = np.asarray(atom2residue_b)
    is_mutation = np.asarray(is_mutation)

    sa = np.searchsorted(node2graph_a, np.arange(G + 1)).astype(np.int64)
    sb = np.searchsorted(node2graph_b, np.arange(G + 1)).astype(np.int64)
    na = np.diff(sa)
    nb = np.diff(sb)
    assert na.min() > 0 and nb.min() > 0, "empty graph block not supported"

    gA = _Geom(na, nb)        # a rows vs b cols
    gB = _Geom(nb, na)        # b rows vs a cols
    key = (gA.key(), gB.key())
    if key not in _prog_cache:
        _prog_cache[key] = _build_program(gA, gB)
    nc = _prog_cache[key]

    in_maps = []
    for c in range(NCORES):
        lhsA, rhsA = _pack_side(gA, pos_a, pos_b, sa, sb, c)
        lhsB, rhsB = _pack_side(gB, pos_b, pos_a, sb, sa, c)
        in_maps.append({"lhsA": lhsA, "rhsA": rhsA,
                        "lhsB": lhsB, "rhsB": rhsB})

    if PROFILE:
        _install_ntff_hook()
    res = run_bass_kernel_spmd(nc, in_maps, list(range(NCORES)),
                               trace=bool(PROFILE))
    if PROFILE:
        LAST_EXEC_NS = res.exec_time_ns

    key_a = np.empty(pos_a.shape[0], np.float64)
    key_b = np.empty(pos_b.shape[0], np.float64)
    for c in range(NCORES):
        vA = res.results[c]["valA"]
        vB = res.results[c]["valB"]
        for s in range(gA.nslots):
            g, r, _ = gA.tile[s][c]
            if g < 0:
                continue
            lo = sa[g] + 128 * r
            hi = min(sa[g] + 128 * (r + 1), sa[g + 1])
            key_a[lo:hi] = vA[s, 0:hi - lo]
        for s in range(gB.nslots):
            g, r, _ = gB.tile[s][c]
            if g < 0:
                continue
            lo = sb[g] + 128 * r
            hi = min(sb[g] + 128 * (r + 1), sb[g + 1])
            key_b[lo:hi] = vB[s, 0:hi - lo]

    # key = max_j (2 a.b_j - |b_j|^2) = |a|^2 - d2_min; |a|^2 exact on host
    d2_a = (pos_a.astype(np.float64) ** 2).sum(-1) - key_a
    d2_b = (pos_b.astype(np.float64) ** 2).sum(-1) - key_b

    def epilogue(d2dev, pos_row, pos_col, s_col, n2row):
        dist = np.sqrt(np.maximum(d2dev, 0.0)).astype(np.float32)
        flags = np.where((dist < 3.0) | (np.abs(dist - 10.0) < 0.6))[0]
        for i in flags:
            g = n2row[i]
            Q = pos_col[s_col[g]:s_col[g + 1]]
            df = pos_row[i].astype(np.float64) - Q.astype(np.float64)
            j = int(np.argmin((df * df).sum(-1)))
            diff = (pos_row[i] - Q[j]).astype(np.float32)
            dist[i] = np.float32(np.sqrt(np.float32((diff * diff).sum())))
        return dist

    dist_a = epilogue(d2_a, pos_a, pos_b, sb, node2graph_a)
    dist_b = epilogue(d2_b, pos_b, pos_a, sa, node2graph_b)

    def iface_mask(dist, atom2residue):
        is_if = (dist < CUTOFF).astype(np.int32)
        res_max = np.zeros(NUM_RESIDUES, dtype=np.int32)
        np.maximum.at(res_max, atom2residue, is_if)
        return res_max[atom2residue] > 0

    mask_a = iface_mask(dist_a, atom2residue_a)
    mask_b = iface_mask(dist_b, atom2residue_b)
    mask = np.concatenate([mask_a, mask_b]) | is_mutation.astype(bool)
    dists = np.concatenate([dist_a, dist_b]).astype(np.float32)
    return mask, dists
